# revision 2
# baseline (speedup 1.0000x reference)
"""Trainium2 Bass kernel for nn_Fine_Change_Moment3.

Math (from the reference):
  - input (16,512,512,16) [b,y,x,t]; fc_weight3 (262144,16,6) per-patch 16x6.
  - Only channel 0 of the CAM survives (cam[:, 0]), so only
    fc_weight3[:, :, 0] matters (host-sliced).
  - Per 4x4 patch n=(b,gy,gx): cam0[t] = sum_k patch[k,t] * w[n,k]
  - v = (cam0 - min_t) / max_t(cam0 - min_t)
  - top[b,t] = v arranged (gy,gx); up = A @ top @ A^T with A the 128->512
    bilinear (align_corners) interp matrix; output (b*512*512, 16) f32.

Distribution: data-parallel over batch, 2 batches per core, 8 cores.

Per-core pipeline:
  1. DMA input rows contiguously: tiles [y=128][(x256,t16)=4096] (16KB/partition runs)
  2. DVE: in-place multiply by per-patch weights (w broadcast over t via
     stride-0 AP), then reduce over px (free axis) -> prod2[p=(gy_l,py)][(gx,t)]
  3. PE: 0/1 selection matmul reduces py across partitions and regroups
     gy -> cam[gy=128][(gx,t)=2048] accumulated in PSUM over 4 y-tiles
  4. DVE: min/max normalize over t (broadcast APs)
  5. PE: 16x transpose 128x128 (per t) -> topT[gx][(t,gy)]
  6. PE f32r: M1[gy][sx] = topT_t^T @ A^T per t, stored interleaved
     M1i[gy][(sx,t)]; then up[sy][(sx32,t16)] = (A^T chunk)^T @ M1i chunk
  7. ACT copies PSUM->SBUF staging [sy][(x,t)] (already t-interleaved),
     contiguous DMA out.
"""

import numpy as np

B, S, T, PP = 16, 512, 16, 4
G = S // PP          # 128 patch grid
NCORES = 8
BPC = B // NCORES    # 2 batches per core

_CACHE = {}


def _interp_matrix_np(n_in, n_out):
    # mirrors the reference's align_corners=True bilinear matrix
    coords = np.arange(n_out, dtype=np.float32) * ((n_in - 1) / (n_out - 1))
    i0 = np.clip(np.floor(coords).astype(np.int64), 0, n_in - 2)
    w = coords - i0.astype(np.float32)
    A = np.zeros((n_out, n_in), dtype=np.float32)
    rows = np.arange(n_out)
    np.add.at(A, (rows, i0), 1.0 - w)
    np.add.at(A, (rows, i0 + 1), w)
    return A  # (n_out, n_in)


def _build_program():
    from contextlib import ExitStack
    import concourse.bacc as bacc
    import concourse.tile as tile
    import concourse.mybir as mybir

    f32 = mybir.dt.float32
    f32r = mybir.dt.float32r
    Alu = mybir.AluOpType
    Ax = mybir.AxisListType

    nc = bacc.Bacc("TRN2", target_bir_lowering=False, debug=False,
                   num_devices=NCORES)

    x_d = nc.dram_tensor("x", [BPC, S, S, T], f32, kind="ExternalInput")
    w_d = nc.dram_tensor("w", [BPC, 4, 128, 512], f32, kind="ExternalInput")
    at_d = nc.dram_tensor("at", [128, 512], f32r, kind="ExternalInput")
    sel_d = nc.dram_tensor("sel", [128, 512], f32r, kind="ExternalInput")
    id_d = nc.dram_tensor("ident", [128, 128], f32, kind="ExternalInput")
    y_d = nc.dram_tensor("y", [BPC, S, S, T], f32, kind="ExternalOutput")

    # input view: [b][yt][xh][y_row=128][(x256 t16)=4096]
    x_v = x_d.ap().rearrange("b (yt p) (xh xx) t -> b yt xh p (xx t)",
                             p=128, xh=2)
    # output view: [b][syc][xh][sy=128][(xx256 t16)=4096]
    y_v = y_d.ap().rearrange("b (syc sy) (xh xx) t -> b syc xh sy (xx t)",
                             syc=4, xh=2)

    with tile.TileContext(nc) as tc, ExitStack() as ctx:
        consts = ctx.enter_context(tc.tile_pool(name="consts", bufs=1))
        pin = ctx.enter_context(tc.tile_pool(name="pin", bufs=3))
        pw = ctx.enter_context(tc.tile_pool(name="pw", bufs=2))
        pp2 = ctx.enter_context(tc.tile_pool(name="pp2", bufs=2))
        pcam = ctx.enter_context(tc.tile_pool(name="pcam", bufs=2))
        pv = ctx.enter_context(tc.tile_pool(name="pv", bufs=2))
        ptop = ctx.enter_context(tc.tile_pool(name="ptop", bufs=2))
        pm1 = ctx.enter_context(tc.tile_pool(name="pm1", bufs=1))
        pst = ctx.enter_context(tc.tile_pool(name="pst", bufs=2))
        pps = ctx.enter_context(tc.tile_pool(name="pps", bufs=2, space="PSUM"))

        at_sb = consts.tile([128, 512], f32r)
        sel_sb = consts.tile([128, 512], f32r)
        id_sb = consts.tile([128, 128], f32)
        nc.sync.dma_start(at_sb[:], at_d.ap())
        nc.sync.dma_start(sel_sb[:], sel_d.ap())
        nc.sync.dma_start(id_sb[:], id_d.ap())

        for b in range(BPC):
            # ---- stage 1+2: weighted patch sums -> prod2 per ytile;
            # ---- stage 3: selection matmuls accumulate cam in PSUM
            cam_ps = pps.tile([128, 2048], f32, tag="ps")
            for yt in range(4):
                w_sb = pw.tile([128, 512], f32, tag="w")
                nc.sync.dma_start(w_sb[:], w_d.ap()[b, yt])
                p2 = pp2.tile([128, 2048], f32r, tag="p2")
                for xh in range(2):
                    it = pin.tile([128, 4096], f32, tag="in")
                    nc.sync.dma_start(it[:], x_v[b, yt, xh])
                    itv = it[:].rearrange("p (gx px t) -> p gx px t",
                                          px=PP, t=T)
                    wv = (w_sb[:, xh * 256:(xh + 1) * 256]
                          .rearrange("p (gx px) -> p gx px", px=PP)
                          .unsqueeze(3).broadcast_to([128, 64, PP, T]))
                    nc.vector.tensor_tensor(itv, itv, wv, op=Alu.mult)
                    rin = it[:].rearrange("p (gx px t) -> p gx t px",
                                          px=PP, t=T)
                    rout = (p2[:, xh * 1024:(xh + 1) * 1024]
                            .rearrange("p (gx t) -> p gx t", t=T))
                    with nc.allow_low_precision(reason="f32r matmul feed"):
                        nc.vector.tensor_reduce(rout, rin, axis=Ax.X,
                                                op=Alu.add)
                for fc in range(4):
                    nc.tensor.matmul(
                        cam_ps[:, fc * 512:(fc + 1) * 512],
                        lhsT=sel_sb[:, yt * 128:(yt + 1) * 128],
                        rhs=p2[:, fc * 512:(fc + 1) * 512],
                        start=(yt == 0), stop=(yt == 3),
                    )

            cam = pcam.tile([128, 2048], f32, tag="cam")
            nc.scalar.copy(cam[:], cam_ps[:])

            # ---- stage 4: normalize over t per (gy, gx)
            v = pv.tile([128, 2048], f32, tag="v")
            mn = pv.tile([128, 128], f32, tag="mn")
            mx = pv.tile([128, 128], f32, tag="mx")
            rx = pv.tile([128, 128], f32, tag="rx")
            cam3 = cam[:].rearrange("p (gx t) -> p gx t", t=T)
            v3 = v[:].rearrange("p (gx t) -> p gx t", t=T)
            nc.vector.tensor_reduce(mn[:], cam3, axis=Ax.X, op=Alu.min)
            mnb = mn[:].unsqueeze(2).broadcast_to([128, 128, T])
            nc.vector.tensor_tensor(v3, cam3, mnb, op=Alu.subtract)
            nc.vector.tensor_reduce(mx[:], v3, axis=Ax.X, op=Alu.max)
            nc.vector.reciprocal(rx[:], mx[:])
            rxb = rx[:].unsqueeze(2).broadcast_to([128, 128, T])
            nc.vector.tensor_tensor(v3, v3, rxb, op=Alu.mult)

            # ---- stage 5: per-t 128x128 transposes -> topT[gx][(t,gy)]
            tp_ps = pps.tile([128, 2048], f32, tag="ps")
            vt = v[:].rearrange("p (gx t) -> p t gx", t=T)
            for t in range(T):
                nc.tensor.transpose(tp_ps[:, t * 128:(t + 1) * 128],
                                    vt[:, t, :], id_sb[:])
            topT = ptop.tile([128, 2048], f32r, tag="top")
            nc.scalar.copy(topT[:], tp_ps[:])

            # ---- stage 6a: M1 = topT_t^T @ AT per t -> M1i[gy][(sx,t)]
            m1i = pm1.tile([128, 8192], f32r, tag="m1i")
            m1iv = m1i[:].rearrange("p (sx t) -> p t sx", t=T)
            for tq in range(4):
                m1_ps = pps.tile([128, 2048], f32, tag="ps")
                for tl in range(4):
                    t = tq * 4 + tl
                    nc.tensor.matmul(
                        m1_ps[:, tl * 512:(tl + 1) * 512],
                        lhsT=topT[:, t * 128:(t + 1) * 128],
                        rhs=at_sb[:],
                        start=True, stop=True,
                    )
                src = m1_ps[:].rearrange("p (tl sx) -> p tl sx", tl=4)
                dst = m1iv[:, tq * 4:(tq + 1) * 4, :]
                nc.scalar.copy(dst, src)

            # ---- stage 6b: up[sy][(sx,t)] chunks; stage 7: staging + DMA out
            for syc in range(4):
                for xh in range(2):
                    stg = pst.tile([128, 4096], f32, tag="stg")
                    for sxg in range(2):
                        up_ps = pps.tile([128, 2048], f32, tag="ps")
                        for sxl in range(4):
                            sxblk = (xh * 2 + sxg) * 4 + sxl
                            nc.tensor.matmul(
                                up_ps[:, sxl * 512:(sxl + 1) * 512],
                                lhsT=at_sb[:, syc * 128:(syc + 1) * 128],
                                rhs=m1i[:, sxblk * 512:(sxblk + 1) * 512],
                                start=True, stop=True,
                            )
                        nc.scalar.copy(stg[:, sxg * 2048:(sxg + 1) * 2048],
                                       up_ps[:])
                    nc.sync.dma_start(y_v[b, syc, xh], stg[:])

    nc.compile()
    return nc


def _host_prep(input, fc_weight3):
    inp = np.ascontiguousarray(input, dtype=np.float32)
    w0 = np.ascontiguousarray(fc_weight3[:, :, 0], dtype=np.float32)
    # w0: (N,16) with n=(b,gy,gx), k=(py,px)
    w0 = w0.reshape(B, 4, 32, G, PP, PP)          # b yt gy_l gx py px
    w_arr = np.ascontiguousarray(
        w0.transpose(0, 1, 2, 4, 3, 5).reshape(B, 4, 128, 512))

    A = _interp_matrix_np(G, S)                   # (512,128)
    at = np.ascontiguousarray(A.T)                # (128,512)

    sel = np.zeros((128, 512), dtype=np.float32)
    p = np.arange(128)
    for j in range(4):
        sel[p, j * 128 + 32 * j + p // 4] = 1.0

    ident = np.eye(128, dtype=np.float32)
    return inp, w_arr, at, sel, ident


def kernel(input, fc_weight3):
    from concourse.bass_utils import run_bass_kernel_spmd

    if "nc" not in _CACHE:
        _CACHE["nc"] = _build_program()
    nc = _CACHE["nc"]

    inp, w_arr, at, sel, ident = _host_prep(input, fc_weight3)

    in_maps = []
    for c in range(NCORES):
        in_maps.append({
            "x": inp[c * BPC:(c + 1) * BPC],
            "w": w_arr[c * BPC:(c + 1) * BPC],
            "at": at,
            "sel": sel,
            "ident": ident,
        })
    res = run_bass_kernel_spmd(nc, in_maps, list(range(NCORES)))
    out = np.concatenate([r["y"] for r in res.results], axis=0)
    return out.reshape(-1, T)


# revision 4
# speedup vs baseline: 1.3204x; 1.3204x over previous
"""Trainium2 Bass kernel for nn_Fine_Change_Moment3.

Math (from the reference):
  - input (16,512,512,16) [b,y,x,t]; fc_weight3 (262144,16,6) per-patch 16x6.
  - Only channel 0 of the CAM survives (cam[:, 0]), so only
    fc_weight3[:, :, 0] matters (host-sliced).
  - Per 4x4 patch n=(b,gy,gx): cam0[t] = sum_k patch[k,t] * w[n,k]
  - v = (cam0 - min_t) / max_t(cam0 - min_t)
  - top[b,t] = v arranged (gy,gx); up = A @ top @ A^T with A the 128->512
    bilinear (align_corners) interp matrix; output (b*512*512, 16) f32.

Distribution: data-parallel over batch, 2 batches per core, 8 cores.

Per-core pipeline:
  1. DMA input rows contiguously: tiles [y=128][(x256,t16)=4096] (16KB/partition runs)
  2. DVE: in-place multiply by per-patch weights (w broadcast over t via
     stride-0 AP), then reduce over px (free axis) -> prod2[p=(gy_l,py)][(gx,t)]
  3. PE: 0/1 selection matmul reduces py across partitions and regroups
     gy -> cam[gy=128][(gx,t)=2048] accumulated in PSUM over 4 y-tiles
  4. DVE: min/max normalize over t (broadcast APs)
  5. PE: 16x transpose 128x128 (per t) -> topT[gx][(t,gy)]
  6. PE f32r: M1[gy][sx] = topT_t^T @ A^T per t, stored interleaved
     M1i[gy][(sx,t)]; then up[sy][(sx32,t16)] = (A^T chunk)^T @ M1i chunk
  7. ACT copies PSUM->SBUF staging [sy][(x,t)] (already t-interleaved),
     contiguous DMA out.
"""

import numpy as np

B, S, T, PP = 16, 512, 16, 4
G = S // PP          # 128 patch grid
NCORES = 8
BPC = B // NCORES    # 2 batches per core

_CACHE = {}


def _interp_matrix_np(n_in, n_out):
    # mirrors the reference's align_corners=True bilinear matrix
    coords = np.arange(n_out, dtype=np.float32) * ((n_in - 1) / (n_out - 1))
    i0 = np.clip(np.floor(coords).astype(np.int64), 0, n_in - 2)
    w = coords - i0.astype(np.float32)
    A = np.zeros((n_out, n_in), dtype=np.float32)
    rows = np.arange(n_out)
    np.add.at(A, (rows, i0), 1.0 - w)
    np.add.at(A, (rows, i0 + 1), w)
    return A  # (n_out, n_in)


def _build_program():
    from contextlib import ExitStack
    import concourse.bacc as bacc
    import concourse.tile as tile
    import concourse.mybir as mybir

    f32 = mybir.dt.float32
    f32r = mybir.dt.float32r
    Alu = mybir.AluOpType
    Ax = mybir.AxisListType

    nc = bacc.Bacc("TRN2", target_bir_lowering=False, debug=False,
                   num_devices=NCORES)

    x_d = nc.dram_tensor("x", [BPC, S, S, T], f32, kind="ExternalInput")
    w_d = nc.dram_tensor("w", [BPC, 4, 128, 512], f32, kind="ExternalInput")
    at_d = nc.dram_tensor("at", [128, 512], f32r, kind="ExternalInput")
    sel_d = nc.dram_tensor("sel", [128, 512], f32r, kind="ExternalInput")
    id_d = nc.dram_tensor("ident", [128, 128], f32, kind="ExternalInput")
    y_d = nc.dram_tensor("y", [BPC, S, S, T], f32, kind="ExternalOutput")

    # input view: [b][yt][xh][y_row=128][(x256 t16)=4096]
    x_v = x_d.ap().rearrange("b (yt p) (xh xx) t -> b yt xh p (xx t)",
                             p=128, xh=2)
    # output view: [b][syc][xh][sy=128][(xx256 t16)=4096]
    y_v = y_d.ap().rearrange("b (syc sy) (xh xx) t -> b syc xh sy (xx t)",
                             syc=4, xh=2)

    with tile.TileContext(nc) as tc, ExitStack() as ctx:
        consts = ctx.enter_context(tc.tile_pool(name="consts", bufs=1))
        pin = ctx.enter_context(tc.tile_pool(name="pin", bufs=3))
        pw = ctx.enter_context(tc.tile_pool(name="pw", bufs=2))
        pp2 = ctx.enter_context(tc.tile_pool(name="pp2", bufs=2))
        pcam = ctx.enter_context(tc.tile_pool(name="pcam", bufs=1))
        pv = ctx.enter_context(tc.tile_pool(name="pv", bufs=1))
        ptop = ctx.enter_context(tc.tile_pool(name="ptop", bufs=1))
        pm1 = ctx.enter_context(tc.tile_pool(name="pm1", bufs=1))
        pst = ctx.enter_context(tc.tile_pool(name="pst", bufs=2))
        ps1 = ctx.enter_context(tc.tile_pool(name="ps1", bufs=2))
        ppsc = ctx.enter_context(tc.tile_pool(name="ppsc", bufs=1,
                                              space="PSUM"))
        ppsw = ctx.enter_context(tc.tile_pool(name="ppsw", bufs=2,
                                              space="PSUM"))

        at_sb = consts.tile([128, 512], f32r)
        sel_sb = consts.tile([128, 512], f32r)
        id_sb = consts.tile([128, 128], f32)
        nc.sync.dma_start(at_sb[:], at_d.ap())
        nc.sync.dma_start(sel_sb[:], sel_d.ap())
        nc.sync.dma_start(id_sb[:], id_d.ap())

        for b in range(BPC):
            # ---- stage 1+2: weighted patch sums -> prod2 per ytile;
            # ---- stage 3: selection matmuls accumulate cam in PSUM
            cam_ps = ppsc.tile([128, 2048], f32, tag="cam")
            for yt in range(4):
                w_sb = pw.tile([128, 512], f32, tag="w")
                nc.sync.dma_start(w_sb[:], w_d.ap()[b, yt])
                p2 = pp2.tile([128, 2048], f32r, tag="p2")
                for xh in range(2):
                    it = pin.tile([128, 4096], f32, tag="in")
                    nc.sync.dma_start(it[:], x_v[b, yt, xh])
                    itv = it[:].rearrange("p (gx px t) -> p gx px t",
                                          px=PP, t=T)
                    wv = (w_sb[:, xh * 256:(xh + 1) * 256]
                          .rearrange("p (gx px) -> p gx px", px=PP)
                          .unsqueeze(3).broadcast_to([128, 64, PP, T]))
                    nc.vector.tensor_tensor(itv, itv, wv, op=Alu.mult)
                    # pairwise add tree over px (contiguous reads)
                    pr = it[:].rearrange("p (gx pxp px2 t) -> p gx pxp px2 t",
                                         pxp=2, px2=2, t=T)
                    s1 = ps1.tile([128, 2048], f32, tag="s1")
                    s1v = s1[:].rearrange("p (gx pxp t) -> p gx pxp t",
                                          pxp=2, t=T)
                    nc.vector.tensor_tensor(s1v, pr[:, :, :, 0, :],
                                            pr[:, :, :, 1, :], op=Alu.add)
                    rout = (p2[:, xh * 1024:(xh + 1) * 1024]
                            .rearrange("p (gx t) -> p gx t", t=T))
                    nc.vector.tensor_tensor(rout, s1v[:, :, 0, :],
                                            s1v[:, :, 1, :], op=Alu.add)
                for fc in range(4):
                    nc.tensor.matmul(
                        cam_ps[:, fc * 512:(fc + 1) * 512],
                        lhsT=sel_sb[:, yt * 128:(yt + 1) * 128],
                        rhs=p2[:, fc * 512:(fc + 1) * 512],
                        start=(yt == 0), stop=(yt == 3),
                    )

            cam = pcam.tile([128, 2048], f32, tag="cam")
            nc.scalar.copy(cam[:], cam_ps[:])

            # ---- stage 4: normalize over t per (gy, gx)
            v = pv.tile([128, 2048], f32, tag="v")
            mn = pv.tile([128, 128], f32, tag="mn")
            mx = pv.tile([128, 128], f32, tag="mx")
            rx = pv.tile([128, 128], f32, tag="rx")
            cam3 = cam[:].rearrange("p (gx t) -> p gx t", t=T)
            v3 = v[:].rearrange("p (gx t) -> p gx t", t=T)
            nc.vector.tensor_reduce(mn[:], cam3, axis=Ax.X, op=Alu.min)
            mnb = mn[:].unsqueeze(2).broadcast_to([128, 128, T])
            nc.vector.tensor_tensor(v3, cam3, mnb, op=Alu.subtract)
            nc.vector.tensor_reduce(mx[:], v3, axis=Ax.X, op=Alu.max)
            nc.vector.reciprocal(rx[:], mx[:])
            rxb = rx[:].unsqueeze(2).broadcast_to([128, 128, T])
            nc.vector.tensor_tensor(v3, v3, rxb, op=Alu.mult)

            # ---- stage 5: per-t 128x128 transposes -> topT[gx][(t,gy)]
            topT = ptop.tile([128, 2048], f32r, tag="top")
            vt = v[:].rearrange("p (gx t) -> p t gx", t=T)
            for th in range(2):
                tp_ps = ppsw.tile([128, 1024], f32, tag="pw")
                for tl in range(8):
                    t = th * 8 + tl
                    nc.tensor.transpose(tp_ps[:, tl * 128:(tl + 1) * 128],
                                        vt[:, t, :], id_sb[:])
                nc.scalar.copy(topT[:, th * 1024:(th + 1) * 1024], tp_ps[:])

            # ---- stage 6a: M1 = topT_t^T @ AT per t -> M1i[gy][(sx,t)]
            m1i = pm1.tile([128, 8192], f32r, tag="m1i")
            m1iv = m1i[:].rearrange("p (sx t) -> p t sx", t=T)
            for tq in range(8):
                m1_ps = ppsw.tile([128, 1024], f32, tag="pw")
                for tl in range(2):
                    t = tq * 2 + tl
                    nc.tensor.matmul(
                        m1_ps[:, tl * 512:(tl + 1) * 512],
                        lhsT=topT[:, t * 128:(t + 1) * 128],
                        rhs=at_sb[:],
                        start=True, stop=True,
                    )
                src = m1_ps[:].rearrange("p (tl sx) -> p tl sx", tl=2)
                dst = m1iv[:, tq * 2:(tq + 1) * 2, :]
                nc.scalar.copy(dst, src)

            # ---- stage 6b: up[sy][(sx,t)] chunks; stage 7: staging + DMA out
            for syc in range(4):
                for xh in range(2):
                    stg = pst.tile([128, 4096], f32, tag="stg")
                    for sxg in range(4):
                        up_ps = ppsw.tile([128, 1024], f32, tag="pw")
                        for sxl in range(2):
                            sxblk = (xh * 4 + sxg) * 2 + sxl
                            nc.tensor.matmul(
                                up_ps[:, sxl * 512:(sxl + 1) * 512],
                                lhsT=at_sb[:, syc * 128:(syc + 1) * 128],
                                rhs=m1i[:, sxblk * 512:(sxblk + 1) * 512],
                                start=True, stop=True,
                            )
                        dst = stg[:, sxg * 1024:(sxg + 1) * 1024]
                        if b == BPC - 1 and sxg % 2 == 1:
                            nc.vector.tensor_copy(dst, up_ps[:])
                        else:
                            nc.scalar.copy(dst, up_ps[:])
                    nc.sync.dma_start(y_v[b, syc, xh], stg[:])

    nc.compile()
    return nc


def _host_prep(input, fc_weight3):
    inp = np.ascontiguousarray(input, dtype=np.float32)
    w0 = np.ascontiguousarray(fc_weight3[:, :, 0], dtype=np.float32)
    # w0: (N,16) with n=(b,gy,gx), k=(py,px)
    w0 = w0.reshape(B, 4, 32, G, PP, PP)          # b yt gy_l gx py px
    w_arr = np.ascontiguousarray(
        w0.transpose(0, 1, 2, 4, 3, 5).reshape(B, 4, 128, 512))

    A = _interp_matrix_np(G, S)                   # (512,128)
    at = np.ascontiguousarray(A.T)                # (128,512)

    sel = np.zeros((128, 512), dtype=np.float32)
    p = np.arange(128)
    for j in range(4):
        sel[p, j * 128 + 32 * j + p // 4] = 1.0

    ident = np.eye(128, dtype=np.float32)
    return inp, w_arr, at, sel, ident


def kernel(input, fc_weight3):
    from concourse.bass_utils import run_bass_kernel_spmd

    if "nc" not in _CACHE:
        _CACHE["nc"] = _build_program()
    nc = _CACHE["nc"]

    inp, w_arr, at, sel, ident = _host_prep(input, fc_weight3)

    in_maps = []
    for c in range(NCORES):
        in_maps.append({
            "x": inp[c * BPC:(c + 1) * BPC],
            "w": w_arr[c * BPC:(c + 1) * BPC],
            "at": at,
            "sel": sel,
            "ident": ident,
        })
    res = run_bass_kernel_spmd(nc, in_maps, list(range(NCORES)))
    out = np.concatenate([r["y"] for r in res.results], axis=0)
    return out.reshape(-1, T)
